# revision 18
# baseline (speedup 1.0000x reference)
"""GAT layer kernel for Trainium2 (Bass/Tile), SPMD over 8 NeuronCores.

Problem (fixed shapes, fp32):
    x: [8, 2048, 128], W: [4, 128, 64], b: [4, 64], a: [4, 128]
    h    = x @ W + b                    (per head)          [B,H,N,64]
    e    = leaky_relu(f_i[:,None] + f_j[None,:], 0.2)       [B,H,N,N]
    attn = softmax(e, axis=-1)
    out  = mean_h(attn @ h)                                 [B,N,64]
  where f_i = h @ a1, f_j = h @ a2.

Sharding: data-parallel — one batch element per core (B == 8 == n_cores).

Math used on-device (exact reformulation):
  exp(leaky(s)) = max(exp(s), exp(0.2 s))  (exp is monotone).
  Softmax over j is invariant to any per-row (per-i) positive scale, so divide
  row i by exp(0.2*c_i):
      Z[j,i] = max( exp(0.8*c_i) * exp(g_j), exp(0.2*g_j) )
  with c = f_i, g = f_j.  Then
      out[i,:] = (sum_j Z[j,i] h[j,:]) / (sum_j Z[j,i]).
  Z is ONE vector-engine tensor_scalar op per tile:
      Z = (v08_bcast * u1[j]) max u2[j]
  where v08_bcast[p, i] = exp(0.8*c_i) broadcast along partitions,
  u1 = exp(g), u2 = exp(0.2*g) are per-partition scalars.

Per core the attention product is computed transposed on the PE:
      oT[o, i] += h_ones[j, o].T @ Z[j, i]   accumulated over j tiles,
  where h_ones = [h_head | 4.0] so row 64 of oT is 4*denominator (the 4 bakes
  in the mean over the 4 heads).  oT is transposed back with the PE, then
  normalized with a reciprocal + per-partition-scalar multiply.
"""

import os
import sys

import numpy as np

_TRN_REPO = "/opt/trn_rl_repo"
if _TRN_REPO not in sys.path and os.path.isdir(_TRN_REPO):
    sys.path.insert(0, _TRN_REPO)

B, N, IN, OUT, H = 8, 2048, 128, 64, 4
NEG_SLOPE = 0.2
NCORES = 8
P = 128  # partition tile

# Column layout of the fused weight matrix WF [IN, H*(OUT+1) + 2*H]:
#   cols h*(OUT+1) .. h*(OUT+1)+OUT-1 : W[h]          -> h values
#   col  h*(OUT+1)+OUT                : zeros (bias 4.0) -> constant 4.0 column
#   col  H*(OUT+1) + h                : W[h] @ a1[h]  -> c = f_i per head
#   col  H*(OUT+1) + H + h            : W[h] @ a2[h]  -> g = f_j per head
HO = OUT + 1          # 65
CBASE = H * HO        # 260
GBASE = H * HO + H    # 264
WCOLS = H * HO + 2 * H  # 268


def _build_program(n=N, attn_f32r=True, repeat=1):
    import concourse.bass as bass
    import concourse.tile as tile
    from concourse import bacc, mybir

    f32 = mybir.dt.float32
    f32r = mybir.dt.float32r
    T = n // P          # node tiles (16)
    IBS = min(512, n)   # i-block size for attn matmuls (one PSUM bank)
    NIB = n // IBS      # i blocks (4)
    hw_dt = f32r if attn_f32r else f32

    nc = bacc.Bacc("TRN2", target_bir_lowering=False, debug=False)

    x_d = nc.dram_tensor("x", [n, IN], f32, kind="ExternalInput")
    wf_d = nc.dram_tensor("wf", [IN, WCOLS], f32, kind="ExternalInput")
    bias_d = nc.dram_tensor("bias", [1, WCOLS], f32, kind="ExternalInput")
    cb08_d = nc.dram_tensor("cb08", [1, H], f32, kind="ExternalInput")
    ones_d = nc.dram_tensor("ones", [1, P], f32, kind="ExternalInput")
    ident_d = nc.dram_tensor("ident", [P, P], f32, kind="ExternalInput")
    out_d = nc.dram_tensor("out", [n, OUT], f32, kind="ExternalOutput")

    Exp = mybir.ActivationFunctionType.Exp
    mult = mybir.AluOpType.mult
    amax = mybir.AluOpType.max
    add = mybir.AluOpType.add

    def body(tc, cst, rep):
        wf_sb, bias_sb, ones_sb, cb08_sb, ident_sb = cst
        with tc.tile_pool(name="bigbuf", bufs=1) as bigpool:
            # x in natural layout, tiled: [128, T*128]; col t*128+i = x[t*128+p, i]
            x_sb = bigpool.tile([P, T * IN], f32, tag="x")
            nc.sync.dma_start(
                x_sb.rearrange("p (t i) -> p t i", t=T),
                x_d.ap().rearrange("(t p) i -> p t i", p=P),
            )

            # ---- transpose x tiles, compute h (+f columns) per node tile ----
            xT_sb = bigpool.tile([P, T * P], f32, tag="xT")  # [i, n]
            h_sb = bigpool.tile([P, T * WCOLS], f32, tag="h")
            # fp32r-rounded copy of the [h | 4.0] weight blocks for the
            # attention matmuls (verifier requires rounded producers)
            hw_sb = bigpool.tile([P, T * CBASE], hw_dt, tag="hw")
            u1_sb = bigpool.tile([P, T * H], f32, tag="u1")  # exp(g)
            u2_sb = bigpool.tile([P, T * H], f32, tag="u2")  # exp(0.2 g)
            # per-head rows [1, n] of exp(0.8 * c), all on partition 0
            e08row_sb = bigpool.tile([1, H * n], f32, tag="e08row")

            with tc.tile_pool(name="setup_ps", bufs=2, space="PSUM") as spool:
                for t in range(T):
                    ps = spool.tile([P, P], f32, tag="xtr")
                    nc.tensor.transpose(
                        ps[:], x_sb[:, t * IN : (t + 1) * IN], ident_sb[:]
                    )
                    nc.scalar.copy(xT_sb[:, t * P : (t + 1) * P], ps[:])
                for t in range(T):
                    ps = spool.tile([P, WCOLS], f32, tag="hmm")
                    # bias broadcast (K=1 matmul), then x.T @ WF accumulated
                    nc.tensor.matmul(
                        ps[:], ones_sb[:], bias_sb[:], start=True, stop=False
                    )
                    nc.tensor.matmul(
                        ps[:],
                        xT_sb[:, t * P : (t + 1) * P],
                        wf_sb[:],
                        start=False,
                        stop=True,
                    )
                    nc.scalar.copy(h_sb[:, t * WCOLS : (t + 1) * WCOLS], ps[:])
                    nc.vector.tensor_copy(
                        hw_sb[:, t * CBASE : (t + 1) * CBASE], ps[:, 0:CBASE]
                    )
                    gcols = ps[:, GBASE : GBASE + H]
                    nc.scalar.activation(
                        u1_sb[:, t * H : (t + 1) * H], gcols, Exp, scale=1.0
                    )
                    nc.scalar.activation(
                        u2_sb[:, t * H : (t + 1) * H], gcols, Exp, scale=0.2
                    )

                # ---- per-head row of exp(0.8*c): c_row = x @ w1_h (M=1) ----
                for h in range(H):
                    for ib in range(NIB):
                        psc = spool.tile([1, IBS], f32, tag="crow")
                        nc.tensor.matmul(
                            psc[:],
                            wf_sb[:, CBASE + h : CBASE + h + 1],
                            xT_sb[:, ib * IBS : (ib + 1) * IBS],
                            start=True,
                            stop=True,
                        )
                        nc.scalar.activation(
                            e08row_sb[0:1, h * n + ib * IBS : h * n + (ib + 1) * IBS],
                            psc[:],
                            Exp,
                            scale=0.8,
                            bias=cb08_sb[0:1, h : h + 1],
                        )

            acc_sb = bigpool.tile([P, T * OUT], f32, tag="acc")

            with (
                tc.tile_pool(name="vbc_ps", bufs=2, space="PSUM") as vbcp,
                tc.tile_pool(name="oT_ps", bufs=4, space="PSUM") as oTp,
                tc.tile_pool(name="tr_ps", bufs=2, space="PSUM") as trp,
                tc.tile_pool(name="vbc", bufs=2) as vbcpool,
                tc.tile_pool(name="z", bufs=3) as zpool,
                tc.tile_pool(name="oTsb", bufs=2) as oTsbpool,
                tc.tile_pool(name="small", bufs=8) as smallpool,
            ):
                for h in range(H):
                    # ---- broadcast exp(0.8 c_h) across partitions ----
                    v08bc = vbcpool.tile([P, n], f32, tag="v08bc")
                    for ib in range(NIB):
                        psb = vbcp.tile([P, IBS], f32, tag="vbc")
                        nc.tensor.matmul(
                            psb[:],
                            ones_sb[:],
                            e08row_sb[0:1, h * n + ib * IBS : h * n + (ib + 1) * IBS],
                            start=True,
                            stop=True,
                        )
                        nc.scalar.copy(v08bc[:, ib * IBS : (ib + 1) * IBS], psb[:])

                    # ---- attention: oT[o, i] += h_ones.T @ Z over j tiles ----
                    oT_ps = [
                        oTp.tile([HO, IBS], f32, tag="oT", name=f"oT_{rep}_{h}_{ib}")
                        for ib in range(NIB)
                    ]
                    for jt in range(T):
                        z = zpool.tile([P, n], hw_dt, tag="z")
                        nc.vector.tensor_scalar(
                            z[:],
                            v08bc[:],
                            u1_sb[:, jt * H + h : jt * H + h + 1],
                            u2_sb[:, jt * H + h : jt * H + h + 1],
                            op0=mult,
                            op1=amax,
                        )
                        lhs = hw_sb[:, jt * CBASE + h * HO : jt * CBASE + (h + 1) * HO]
                        for ib in range(NIB):
                            nc.tensor.matmul(
                                oT_ps[ib][:],
                                lhs,
                                z[:, ib * IBS : (ib + 1) * IBS],
                                start=(jt == 0),
                                stop=(jt == T - 1),
                            )

                    # ---- transpose back, normalize, accumulate over heads ----
                    oT_sb = oTsbpool.tile([HO, n], f32, tag="oTsb")
                    for ib in range(NIB):
                        nc.scalar.copy(
                            oT_sb[:, ib * IBS : (ib + 1) * IBS], oT_ps[ib][:]
                        )
                    for it in range(T):
                        pst = trp.tile([P, HO], f32, tag="otr")
                        nc.tensor.transpose(
                            pst[:],
                            oT_sb[:, it * P : (it + 1) * P],
                            ident_sb[0:HO, 0:HO],
                        )
                        rec = smallpool.tile([P, 1], f32, tag="rec")
                        nc.vector.reciprocal(rec[:], pst[:, OUT : OUT + 1])
                        accsl = acc_sb[:, it * OUT : (it + 1) * OUT]
                        if h == 0:
                            nc.vector.tensor_scalar(
                                accsl, pst[:, 0:OUT], rec[:], None, op0=mult
                            )
                        else:
                            nc.vector.scalar_tensor_tensor(
                                accsl, pst[:, 0:OUT], rec[:], accsl,
                                op0=mult, op1=add,
                            )

            nc.sync.dma_start(
                out_d.ap().rearrange("(t p) o -> p t o", p=P),
                acc_sb.rearrange("p (t o) -> p t o", t=T),
            )

    with tile.TileContext(nc) as tc:
        with tc.tile_pool(name="const", bufs=1) as cpool:
            wf_sb = cpool.tile([IN, WCOLS], f32, tag="wf")
            nc.sync.dma_start(wf_sb[:], wf_d.ap())
            bias_sb = cpool.tile([1, WCOLS], f32, tag="bias")
            nc.sync.dma_start(bias_sb[:], bias_d.ap())
            ones_sb = cpool.tile([1, P], f32, tag="ones")
            nc.sync.dma_start(ones_sb[:], ones_d.ap())
            cb08_sb = cpool.tile([1, H], f32, tag="cb08")
            nc.sync.dma_start(cb08_sb[:], cb08_d.ap())
            ident_sb = cpool.tile([P, P], f32, tag="ident")
            nc.sync.dma_start(ident_sb[:], ident_d.ap())

            cst = (wf_sb, bias_sb, ones_sb, cb08_sb, ident_sb)
            for rep in range(repeat):
                body(tc, cst, rep)

    nc.compile()
    return nc


def _prep_params(W, b, a):
    W = np.asarray(W, np.float32)
    b = np.asarray(b, np.float32)
    a = np.asarray(a, np.float32)
    a1, a2 = a[:, :OUT], a[:, OUT:]
    wf = np.zeros((IN, WCOLS), np.float32)
    bias = np.zeros((1, WCOLS), np.float32)
    cb08 = np.zeros((1, H), np.float32)
    for h in range(H):
        wf[:, h * HO : h * HO + OUT] = W[h]
        bias[0, h * HO : h * HO + OUT] = b[h]
        bias[0, h * HO + OUT] = float(H)  # denominator scale -> head mean
        wf[:, CBASE + h] = W[h] @ a1[h]
        bias[0, CBASE + h] = float(b[h] @ a1[h])
        wf[:, GBASE + h] = W[h] @ a2[h]
        bias[0, GBASE + h] = float(b[h] @ a2[h])
        cb08[0, h] = 0.8 * float(b[h] @ a1[h])
    return wf, bias, cb08


def _make_in_maps(x, W, b, a):
    wf, bias, cb08 = _prep_params(W, b, a)
    ones = np.ones((1, P), np.float32)
    ident = np.eye(P, dtype=np.float32)
    return [
        {"x": np.ascontiguousarray(x[i]), "wf": wf, "bias": bias, "cb08": cb08,
         "ones": ones, "ident": ident}
        for i in range(NCORES)
    ]


_PROGRAM = None


def kernel(x, W, b, a):
    global _PROGRAM
    from concourse import bass_utils

    x = np.asarray(x, np.float32)
    assert x.shape == (B, N, IN), x.shape

    if _PROGRAM is None:
        _PROGRAM = _build_program()
    nc = _PROGRAM

    in_maps = _make_in_maps(x, W, b, a)
    res = bass_utils.run_bass_kernel_spmd(nc, in_maps, core_ids=list(range(NCORES)))
    out = np.stack([res.results[i]["out"] for i in range(NCORES)], axis=0)
    return out.astype(np.float32)


# revision 20
# speedup vs baseline: 14.7024x; 14.7024x over previous
"""GAT layer kernel for Trainium2 (Bass/Tile), SPMD over 8 NeuronCores.

Problem (fixed shapes, fp32):
    x: [8, 2048, 128], W: [4, 128, 64], b: [4, 64], a: [4, 128]
    h    = x @ W + b                    (per head)          [B,H,N,64]
    e    = leaky_relu(f_i[:,None] + f_j[None,:], 0.2)       [B,H,N,N]
    attn = softmax(e, axis=-1)
    out  = mean_h(attn @ h)                                 [B,N,64]
  where f_i = h @ a1, f_j = h @ a2.

Sharding: data-parallel — one batch element per core (B == 8 == n_cores).

Math used on-device (exact reformulation):
  exp(leaky(s)) = max(exp(s), exp(0.2 s))  (exp is monotone).
  Softmax over j is invariant to any per-row (per-i) positive scale, so divide
  row i by exp(0.2*c_i):
      Z[j,i] = max( exp(0.8*c_i) * exp(g_j), exp(0.2*g_j) )
  with c = f_i, g = f_j.  Then
      out[i,:] = (sum_j Z[j,i] h[j,:]) / (sum_j Z[j,i]).
  Z is ONE vector-engine tensor_scalar op per tile:
      Z = (v08_bcast * u1[j]) max u2[j]
  where v08_bcast[p, i] = exp(0.8*c_i) broadcast along partitions,
  u1 = exp(g), u2 = exp(0.2*g) are per-partition scalars.

Per core the attention product is computed transposed on the PE:
      oT[o, i] += h_ones[j, o].T @ Z[j, i]   accumulated over j tiles,
  where h_ones = [h_head | 4.0] so row 64 of oT is 4*denominator (the 4 bakes
  in the mean over the 4 heads).  oT is transposed back with the PE, then
  normalized with a reciprocal + per-partition-scalar multiply.
"""

import os
import sys

import numpy as np

_TRN_REPO = "/opt/trn_rl_repo"
if _TRN_REPO not in sys.path and os.path.isdir(_TRN_REPO):
    sys.path.insert(0, _TRN_REPO)

B, N, IN, OUT, H = 8, 2048, 128, 64, 4
NEG_SLOPE = 0.2
NCORES = 8
P = 128  # partition tile

# Column layout of the fused weight matrix WF [IN, H*(OUT+1) + 2*H]:
#   cols h*(OUT+1) .. h*(OUT+1)+OUT-1 : W[h]          -> h values
#   col  h*(OUT+1)+OUT                : zeros (bias 4.0) -> constant 4.0 column
#   col  H*(OUT+1) + h                : W[h] @ a1[h]  -> c = f_i per head
#   col  H*(OUT+1) + H + h            : W[h] @ a2[h]  -> g = f_j per head
HO = OUT + 1          # 65
CBASE = H * HO        # 260
GBASE = H * HO + H    # 264
WCOLS = H * HO + 2 * H  # 268


def _build_program(n=N, attn_f32r=True, repeat=1, hw_loop=0):
    import concourse.bass as bass
    import concourse.tile as tile
    from concourse import bacc, mybir

    f32 = mybir.dt.float32
    f32r = mybir.dt.float32r
    T = n // P          # node tiles (16)
    IBS = min(512, n)   # i-block size for attn matmuls (one PSUM bank)
    NIB = n // IBS      # i blocks (4)
    hw_dt = f32r if attn_f32r else f32

    nc = bacc.Bacc("TRN2", target_bir_lowering=False, debug=False)

    x_d = nc.dram_tensor("x", [n, IN], f32, kind="ExternalInput")
    wf_d = nc.dram_tensor("wf", [IN, WCOLS], f32, kind="ExternalInput")
    bias_d = nc.dram_tensor("bias", [1, WCOLS], f32, kind="ExternalInput")
    cb08_d = nc.dram_tensor("cb08", [1, H], f32, kind="ExternalInput")
    ones_d = nc.dram_tensor("ones", [1, P], f32, kind="ExternalInput")
    ident_d = nc.dram_tensor("ident", [P, P], f32, kind="ExternalInput")
    out_d = nc.dram_tensor("out", [n, OUT], f32, kind="ExternalOutput")

    Exp = mybir.ActivationFunctionType.Exp
    mult = mybir.AluOpType.mult
    amax = mybir.AluOpType.max
    add = mybir.AluOpType.add

    def body(tc, cst, rep):
        wf_sb, bias_sb, ones_sb, cb08_sb, ident_sb = cst
        with tc.tile_pool(name="bigbuf", bufs=1) as bigpool:
            # x in natural layout, tiled: [128, T*128]; col t*128+i = x[t*128+p, i]
            x_sb = bigpool.tile([P, T * IN], f32, tag="x")
            nc.sync.dma_start(
                x_sb.rearrange("p (t i) -> p t i", t=T),
                x_d.ap().rearrange("(t p) i -> p t i", p=P),
            )

            # ---- transpose x tiles, compute h (+f columns) per node tile ----
            xT_sb = bigpool.tile([P, T * P], f32, tag="xT")  # [i, n]
            h_sb = bigpool.tile([P, T * WCOLS], f32, tag="h")
            # fp32r-rounded copy of the [h | 4.0] weight blocks for the
            # attention matmuls (verifier requires rounded producers)
            hw_sb = bigpool.tile([P, T * CBASE], hw_dt, tag="hw")
            u1_sb = bigpool.tile([P, T * H], f32, tag="u1")  # exp(g)
            u2_sb = bigpool.tile([P, T * H], f32, tag="u2")  # exp(0.2 g)
            # per-head rows [1, n] of exp(0.8 * c), all on partition 0
            e08row_sb = bigpool.tile([1, H * n], f32, tag="e08row")

            with tc.tile_pool(name="setup_ps", bufs=2, space="PSUM") as spool:
                for t in range(T):
                    ps = spool.tile([P, P], f32, tag="xtr")
                    nc.tensor.transpose(
                        ps[:], x_sb[:, t * IN : (t + 1) * IN], ident_sb[:]
                    )
                    nc.scalar.copy(xT_sb[:, t * P : (t + 1) * P], ps[:])
                for t in range(T):
                    ps = spool.tile([P, WCOLS], f32, tag="hmm")
                    # bias broadcast (K=1 matmul), then x.T @ WF accumulated
                    nc.tensor.matmul(
                        ps[:], ones_sb[:], bias_sb[:], start=True, stop=False
                    )
                    nc.tensor.matmul(
                        ps[:],
                        xT_sb[:, t * P : (t + 1) * P],
                        wf_sb[:],
                        start=False,
                        stop=True,
                    )
                    nc.scalar.copy(h_sb[:, t * WCOLS : (t + 1) * WCOLS], ps[:])
                    nc.vector.tensor_copy(
                        hw_sb[:, t * CBASE : (t + 1) * CBASE], ps[:, 0:CBASE]
                    )
                    gcols = ps[:, GBASE : GBASE + H]
                    nc.scalar.activation(
                        u1_sb[:, t * H : (t + 1) * H], gcols, Exp, scale=1.0
                    )
                    nc.scalar.activation(
                        u2_sb[:, t * H : (t + 1) * H], gcols, Exp, scale=0.2
                    )

                # ---- per-head row of exp(0.8*c): c_row = x @ w1_h (M=1) ----
                for h in range(H):
                    for ib in range(NIB):
                        psc = spool.tile([1, IBS], f32, tag="crow")
                        nc.tensor.matmul(
                            psc[:],
                            wf_sb[:, CBASE + h : CBASE + h + 1],
                            xT_sb[:, ib * IBS : (ib + 1) * IBS],
                            start=True,
                            stop=True,
                        )
                        nc.scalar.activation(
                            e08row_sb[0:1, h * n + ib * IBS : h * n + (ib + 1) * IBS],
                            psc[:],
                            Exp,
                            scale=0.8,
                            bias=cb08_sb[0:1, h : h + 1],
                        )

            acc_sb = bigpool.tile([P, T * OUT], f32, tag="acc")

            with (
                tc.tile_pool(name="vbc_ps", bufs=2, space="PSUM") as vbcp,
                tc.tile_pool(name="oT_ps", bufs=4, space="PSUM") as oTp,
                tc.tile_pool(name="tr_ps", bufs=2, space="PSUM") as trp,
                tc.tile_pool(name="vbc", bufs=2) as vbcpool,
                tc.tile_pool(name="z", bufs=3) as zpool,
                tc.tile_pool(name="oTsb", bufs=2) as oTsbpool,
                tc.tile_pool(name="small", bufs=8) as smallpool,
            ):
                for h in range(H):
                    # ---- broadcast exp(0.8 c_h) across partitions ----
                    v08bc = vbcpool.tile([P, n], f32, tag="v08bc")
                    for ib in range(NIB):
                        psb = vbcp.tile([P, IBS], f32, tag="vbc")
                        nc.tensor.matmul(
                            psb[:],
                            ones_sb[:],
                            e08row_sb[0:1, h * n + ib * IBS : h * n + (ib + 1) * IBS],
                            start=True,
                            stop=True,
                        )
                        nc.scalar.copy(v08bc[:, ib * IBS : (ib + 1) * IBS], psb[:])

                    # ---- attention: oT[o, i] += h_ones.T @ Z over j tiles ----
                    oT_ps = [
                        oTp.tile([HO, IBS], f32, tag="oT", name=f"oT_{rep}_{h}_{ib}")
                        for ib in range(NIB)
                    ]
                    for jt in range(T):
                        z = zpool.tile([P, n], hw_dt, tag="z")
                        nc.vector.tensor_scalar(
                            z[:],
                            v08bc[:],
                            u1_sb[:, jt * H + h : jt * H + h + 1],
                            u2_sb[:, jt * H + h : jt * H + h + 1],
                            op0=mult,
                            op1=amax,
                        )
                        lhs = hw_sb[:, jt * CBASE + h * HO : jt * CBASE + (h + 1) * HO]
                        for ib in range(NIB):
                            nc.tensor.matmul(
                                oT_ps[ib][:],
                                lhs,
                                z[:, ib * IBS : (ib + 1) * IBS],
                                start=(jt == 0),
                                stop=(jt == T - 1),
                            )

                    # ---- transpose back, normalize, accumulate over heads ----
                    oT_sb = oTsbpool.tile([HO, n], f32, tag="oTsb")
                    for ib in range(NIB):
                        nc.scalar.copy(
                            oT_sb[:, ib * IBS : (ib + 1) * IBS], oT_ps[ib][:]
                        )
                    for it in range(T):
                        pst = trp.tile([P, HO], f32, tag="otr")
                        nc.tensor.transpose(
                            pst[:],
                            oT_sb[:, it * P : (it + 1) * P],
                            ident_sb[0:HO, 0:HO],
                        )
                        rec = smallpool.tile([P, 1], f32, tag="rec")
                        nc.vector.reciprocal(rec[:], pst[:, OUT : OUT + 1])
                        accsl = acc_sb[:, it * OUT : (it + 1) * OUT]
                        if h == 0:
                            nc.vector.tensor_scalar(
                                accsl, pst[:, 0:OUT], rec[:], None, op0=mult
                            )
                        else:
                            nc.vector.scalar_tensor_tensor(
                                accsl, pst[:, 0:OUT], rec[:], accsl,
                                op0=mult, op1=add,
                            )

            nc.sync.dma_start(
                out_d.ap().rearrange("(t p) o -> p t o", p=P),
                acc_sb.rearrange("p (t o) -> p t o", t=T),
            )

    with tile.TileContext(nc) as tc:
        with tc.tile_pool(name="const", bufs=1) as cpool:
            wf_sb = cpool.tile([IN, WCOLS], f32, tag="wf")
            nc.sync.dma_start(wf_sb[:], wf_d.ap())
            bias_sb = cpool.tile([1, WCOLS], f32, tag="bias")
            nc.sync.dma_start(bias_sb[:], bias_d.ap())
            ones_sb = cpool.tile([1, P], f32, tag="ones")
            nc.sync.dma_start(ones_sb[:], ones_d.ap())
            cb08_sb = cpool.tile([1, H], f32, tag="cb08")
            nc.sync.dma_start(cb08_sb[:], cb08_d.ap())
            ident_sb = cpool.tile([P, P], f32, tag="ident")
            nc.sync.dma_start(ident_sb[:], ident_d.ap())

            cst = (wf_sb, bias_sb, ones_sb, cb08_sb, ident_sb)
            if hw_loop:
                # hardware loop: body emitted once, looped on-device (used
                # for amortized timing measurements)
                with tc.For_i(0, hw_loop, 1):
                    body(tc, cst, 0)
            else:
                for rep in range(repeat):
                    body(tc, cst, rep)

    nc.compile()
    return nc


def _prep_params(W, b, a):
    W = np.asarray(W, np.float32)
    b = np.asarray(b, np.float32)
    a = np.asarray(a, np.float32)
    a1, a2 = a[:, :OUT], a[:, OUT:]
    wf = np.zeros((IN, WCOLS), np.float32)
    bias = np.zeros((1, WCOLS), np.float32)
    cb08 = np.zeros((1, H), np.float32)
    for h in range(H):
        wf[:, h * HO : h * HO + OUT] = W[h]
        bias[0, h * HO : h * HO + OUT] = b[h]
        bias[0, h * HO + OUT] = float(H)  # denominator scale -> head mean
        wf[:, CBASE + h] = W[h] @ a1[h]
        bias[0, CBASE + h] = float(b[h] @ a1[h])
        wf[:, GBASE + h] = W[h] @ a2[h]
        bias[0, GBASE + h] = float(b[h] @ a2[h])
        cb08[0, h] = 0.8 * float(b[h] @ a1[h])
    return wf, bias, cb08


def _make_in_maps(x, W, b, a):
    wf, bias, cb08 = _prep_params(W, b, a)
    ones = np.ones((1, P), np.float32)
    ident = np.eye(P, dtype=np.float32)
    return [
        {"x": np.ascontiguousarray(x[i]), "wf": wf, "bias": bias, "cb08": cb08,
         "ones": ones, "ident": ident}
        for i in range(NCORES)
    ]


_PROGRAM = None


def kernel(x, W, b, a):
    global _PROGRAM
    from concourse import bass_utils

    x = np.asarray(x, np.float32)
    assert x.shape == (B, N, IN), x.shape

    if _PROGRAM is None:
        _PROGRAM = _build_program()
    nc = _PROGRAM

    in_maps = _make_in_maps(x, W, b, a)
    res = bass_utils.run_bass_kernel_spmd(nc, in_maps, core_ids=list(range(NCORES)))
    out = np.stack([res.results[i]["out"] for i in range(NCORES)], axis=0)
    return out.astype(np.float32)


# revision 24
# speedup vs baseline: 34.0547x; 2.3163x over previous
"""GAT layer kernel for Trainium2 (Bass/Tile), SPMD over 8 NeuronCores.

Problem (fixed shapes, fp32):
    x: [8, 2048, 128], W: [4, 128, 64], b: [4, 64], a: [4, 128]
    h    = x @ W + b                    (per head)          [B,H,N,64]
    e    = leaky_relu(f_i[:,None] + f_j[None,:], 0.2)       [B,H,N,N]
    attn = softmax(e, axis=-1)
    out  = mean_h(attn @ h)                                 [B,N,64]
  where f_i = h @ a1, f_j = h @ a2.

Sharding: data-parallel — one batch element per core (B == 8 == n_cores).

Math used on-device (exact reformulation):
  exp(leaky(s)) = max(exp(s), exp(0.2 s))  (exp is monotone).
  Softmax over j is invariant to any per-row (per-i) positive scale, so divide
  row i by exp(0.2*c_i):
      Z[j,i] = max( exp(0.8*c_i) * exp(g_j), exp(0.2*g_j) )
  with c = f_i, g = f_j.  Then
      out[i,:] = (sum_j Z[j,i] h[j,:]) / (sum_j Z[j,i]).
  Z is ONE vector-engine tensor_scalar op per tile:
      Z = (v08_bcast * u1[j]) max u2[j]
  where v08_bcast[p, i] = exp(0.8*c_i) broadcast along partitions,
  u1 = exp(g), u2 = exp(0.2*g) are per-partition scalars.

Per core the attention product is computed transposed on the PE:
      oT[o, i] += h_ones[j, o].T @ Z[j, i]   accumulated over j tiles,
  where h_ones = [h_head | 4.0] so row 64 of oT is 4*denominator (the 4 bakes
  in the mean over the 4 heads).  oT is transposed back with the PE, then
  normalized with a reciprocal + per-partition-scalar multiply.
"""

import os
import sys

import numpy as np

_TRN_REPO = "/opt/trn_rl_repo"
if _TRN_REPO not in sys.path and os.path.isdir(_TRN_REPO):
    sys.path.insert(0, _TRN_REPO)

B, N, IN, OUT, H = 8, 2048, 128, 64, 4
NEG_SLOPE = 0.2
NCORES = 8
P = 128  # partition tile

# Column layout of the fused weight matrix WF [IN, H*(OUT+1) + 2*H]:
#   cols h*(OUT+1) .. h*(OUT+1)+OUT-1 : W[h]          -> h values
#   col  h*(OUT+1)+OUT                : zeros (bias 4.0) -> constant 4.0 column
#   col  H*(OUT+1) + h                : W[h] @ a1[h]  -> c = f_i per head
#   col  H*(OUT+1) + H + h            : W[h] @ a2[h]  -> g = f_j per head
HO = OUT + 1          # 65
CBASE = H * HO        # 260
GBASE = H * HO + H    # 264
WCOLS = H * HO + 2 * H  # 268


def _build_program(n=N, attn_f32r=True, repeat=1, hw_loop=0):
    import concourse.bass as bass
    import concourse.tile as tile
    from concourse import bacc, mybir

    f32 = mybir.dt.float32
    f32r = mybir.dt.float32r
    T = n // P          # node tiles (16)
    IBS = min(512, n)   # i-block size for attn matmuls (one PSUM bank)
    NIB = n // IBS      # i blocks (4)
    hw_dt = f32r if attn_f32r else f32

    nc = bacc.Bacc("TRN2", target_bir_lowering=False, debug=False)

    x_d = nc.dram_tensor("x", [n, IN], f32, kind="ExternalInput")
    wf_d = nc.dram_tensor("wf", [IN, WCOLS], f32, kind="ExternalInput")
    bias_d = nc.dram_tensor("bias", [1, WCOLS], f32, kind="ExternalInput")
    cb08_d = nc.dram_tensor("cb08", [1, H], f32, kind="ExternalInput")
    ones_d = nc.dram_tensor("ones", [1, P], f32, kind="ExternalInput")
    ident_d = nc.dram_tensor("ident", [P, P], f32, kind="ExternalInput")
    out_d = nc.dram_tensor("out", [n, OUT], f32, kind="ExternalOutput")
    # internal DRAM bounce buffer for the partition-broadcast of exp(0.8c)
    e08s_d = nc.dram_tensor("e08scratch", [1, H * n], f32)

    Exp = mybir.ActivationFunctionType.Exp
    mult = mybir.AluOpType.mult
    amax = mybir.AluOpType.max
    add = mybir.AluOpType.add

    def body(tc, cst, rep):
        wf_sb, bias_sb, ones_sb, cb08_sb, ident_sb = cst
        with tc.tile_pool(name="bigbuf", bufs=1) as bigpool:
            # x in natural layout, tiled: [128, T*128]; col t*128+i = x[t*128+p, i]
            x_sb = bigpool.tile([P, T * IN], f32, tag="x")
            nc.sync.dma_start(
                x_sb.rearrange("p (t i) -> p t i", t=T),
                x_d.ap().rearrange("(t p) i -> p t i", p=P),
            )

            # ---- transpose x tiles, compute h (+f columns) per node tile ----
            xT_sb = bigpool.tile([P, T * P], f32, tag="xT")  # [i, n]
            h_sb = bigpool.tile([P, T * WCOLS], f32, tag="h")
            # fp32r-rounded copy of the [h | 4.0] weight blocks for the
            # attention matmuls (verifier requires rounded producers)
            hw_sb = bigpool.tile([P, T * CBASE], hw_dt, tag="hw")
            u1_sb = bigpool.tile([P, T * H], f32, tag="u1")  # exp(g)
            u2_sb = bigpool.tile([P, T * H], f32, tag="u2")  # exp(0.2 g)
            # per-head rows [1, n] of exp(0.8 * c), all on partition 0
            e08row_sb = bigpool.tile([1, H * n], f32, tag="e08row")

            with tc.tile_pool(name="setup_ps", bufs=2, space="PSUM") as spool:
                for t in range(T):
                    ps = spool.tile([P, P], f32, tag="xtr")
                    nc.tensor.transpose(
                        ps[:], x_sb[:, t * IN : (t + 1) * IN], ident_sb[:]
                    )
                    nc.scalar.copy(xT_sb[:, t * P : (t + 1) * P], ps[:])
                for t in range(T):
                    ps = spool.tile([P, WCOLS], f32, tag="hmm")
                    # bias broadcast (K=1 matmul), then x.T @ WF accumulated
                    nc.tensor.matmul(
                        ps[:], ones_sb[:], bias_sb[:], start=True, stop=False
                    )
                    nc.tensor.matmul(
                        ps[:],
                        xT_sb[:, t * P : (t + 1) * P],
                        wf_sb[:],
                        start=False,
                        stop=True,
                    )
                    nc.scalar.copy(h_sb[:, t * WCOLS : (t + 1) * WCOLS], ps[:])
                    nc.vector.tensor_copy(
                        hw_sb[:, t * CBASE : (t + 1) * CBASE], ps[:, 0:CBASE]
                    )
                    gcols = ps[:, GBASE : GBASE + H]
                    nc.scalar.activation(
                        u1_sb[:, t * H : (t + 1) * H], gcols, Exp, scale=1.0
                    )
                    nc.scalar.activation(
                        u2_sb[:, t * H : (t + 1) * H], gcols, Exp, scale=0.2
                    )

                # ---- per-head row of exp(0.8*c): c_row = x @ w1_h (M=1) ----
                for h in range(H):
                    for ib in range(NIB):
                        psc = spool.tile([1, IBS], f32, tag="crow")
                        nc.tensor.matmul(
                            psc[:],
                            wf_sb[:, CBASE + h : CBASE + h + 1],
                            xT_sb[:, ib * IBS : (ib + 1) * IBS],
                            start=True,
                            stop=True,
                        )
                        nc.scalar.activation(
                            e08row_sb[0:1, h * n + ib * IBS : h * n + (ib + 1) * IBS],
                            psc[:],
                            Exp,
                            scale=0.8,
                            bias=cb08_sb[0:1, h : h + 1],
                        )

            # bounce exp(0.8c) rows through DRAM so they can be DMA-broadcast
            # across partitions (stride-0 reads are only legal on DRAM APs)
            nc.sync.dma_start(e08s_d.ap(), e08row_sb[:])

            acc_sb = bigpool.tile([P, T * OUT], f32, tag="acc")

            with (
                tc.tile_pool(name="oT_ps", bufs=4, space="PSUM") as oTp,
                tc.tile_pool(name="tr_ps", bufs=2, space="PSUM") as trp,
                tc.tile_pool(name="vbc", bufs=2) as vbcpool,
                tc.tile_pool(name="z", bufs=3) as zpool,
                tc.tile_pool(name="oTsb", bufs=2) as oTsbpool,
                tc.tile_pool(name="small", bufs=8) as smallpool,
            ):
                for h in range(H):
                    # ---- broadcast exp(0.8 c_h) across partitions via DMA ----
                    v08bc = vbcpool.tile([P, n], f32, tag="v08bc")
                    for ib in range(NIB):
                        sl = e08s_d.ap()[0:1, h * n + ib * IBS : h * n + (ib + 1) * IBS]
                        bcast_ap = bass.AP(
                            tensor=sl.tensor, offset=sl.offset,
                            ap=[[0, P]] + sl.ap[1:],
                        )
                        nc.sync.dma_start(
                            v08bc[:, ib * IBS : (ib + 1) * IBS], bcast_ap
                        )

                    # ---- attention: oT[o, i] += h_ones.T @ Z over j tiles ----
                    oT_ps = [
                        oTp.tile([HO, IBS], f32, tag="oT", name=f"oT_{rep}_{h}_{ib}")
                        for ib in range(NIB)
                    ]
                    for jt in range(T):
                        z = zpool.tile([P, n], hw_dt, tag="z")
                        # split the big elementwise pass between DVE and GPSIMD
                        zeng = nc.gpsimd if jt % 3 == 2 else nc.vector
                        zeng.tensor_scalar(
                            z[:],
                            v08bc[:],
                            u1_sb[:, jt * H + h : jt * H + h + 1],
                            u2_sb[:, jt * H + h : jt * H + h + 1],
                            op0=mult,
                            op1=amax,
                        )
                        lhs = hw_sb[:, jt * CBASE + h * HO : jt * CBASE + (h + 1) * HO]
                        for ib in range(NIB):
                            nc.tensor.matmul(
                                oT_ps[ib][:],
                                lhs,
                                z[:, ib * IBS : (ib + 1) * IBS],
                                start=(jt == 0),
                                stop=(jt == T - 1),
                            )

                    # ---- transpose back, normalize, accumulate over heads ----
                    oT_sb = oTsbpool.tile([HO, n], f32, tag="oTsb")
                    for ib in range(NIB):
                        nc.scalar.copy(
                            oT_sb[:, ib * IBS : (ib + 1) * IBS], oT_ps[ib][:]
                        )
                    for it in range(T):
                        pst = trp.tile([P, HO], f32, tag="otr")
                        nc.tensor.transpose(
                            pst[:],
                            oT_sb[:, it * P : (it + 1) * P],
                            ident_sb[0:HO, 0:HO],
                        )
                        rec = smallpool.tile([P, 1], f32, tag="rec")
                        nc.vector.reciprocal(rec[:], pst[:, OUT : OUT + 1])
                        accsl = acc_sb[:, it * OUT : (it + 1) * OUT]
                        if h == 0:
                            nc.vector.tensor_scalar(
                                accsl, pst[:, 0:OUT], rec[:], None, op0=mult
                            )
                        else:
                            nc.vector.scalar_tensor_tensor(
                                accsl, pst[:, 0:OUT], rec[:], accsl,
                                op0=mult, op1=add,
                            )

            nc.sync.dma_start(
                out_d.ap().rearrange("(t p) o -> p t o", p=P),
                acc_sb.rearrange("p (t o) -> p t o", t=T),
            )

    with tile.TileContext(nc) as tc:
        with tc.tile_pool(name="const", bufs=1) as cpool:
            wf_sb = cpool.tile([IN, WCOLS], f32, tag="wf")
            nc.sync.dma_start(wf_sb[:], wf_d.ap())
            bias_sb = cpool.tile([1, WCOLS], f32, tag="bias")
            nc.sync.dma_start(bias_sb[:], bias_d.ap())
            ones_sb = cpool.tile([1, P], f32, tag="ones")
            nc.sync.dma_start(ones_sb[:], ones_d.ap())
            cb08_sb = cpool.tile([1, H], f32, tag="cb08")
            nc.sync.dma_start(cb08_sb[:], cb08_d.ap())
            ident_sb = cpool.tile([P, P], f32, tag="ident")
            nc.sync.dma_start(ident_sb[:], ident_d.ap())

            cst = (wf_sb, bias_sb, ones_sb, cb08_sb, ident_sb)
            if hw_loop:
                # hardware loop: body emitted once, looped on-device (used
                # for amortized timing measurements)
                with tc.For_i(0, hw_loop, 1):
                    body(tc, cst, 0)
            else:
                for rep in range(repeat):
                    body(tc, cst, rep)

    nc.compile()
    return nc


def _prep_params(W, b, a):
    W = np.asarray(W, np.float32)
    b = np.asarray(b, np.float32)
    a = np.asarray(a, np.float32)
    a1, a2 = a[:, :OUT], a[:, OUT:]
    wf = np.zeros((IN, WCOLS), np.float32)
    bias = np.zeros((1, WCOLS), np.float32)
    cb08 = np.zeros((1, H), np.float32)
    for h in range(H):
        wf[:, h * HO : h * HO + OUT] = W[h]
        bias[0, h * HO : h * HO + OUT] = b[h]
        bias[0, h * HO + OUT] = float(H)  # denominator scale -> head mean
        wf[:, CBASE + h] = W[h] @ a1[h]
        bias[0, CBASE + h] = float(b[h] @ a1[h])
        wf[:, GBASE + h] = W[h] @ a2[h]
        bias[0, GBASE + h] = float(b[h] @ a2[h])
        cb08[0, h] = 0.8 * float(b[h] @ a1[h])
    return wf, bias, cb08


def _make_in_maps(x, W, b, a):
    wf, bias, cb08 = _prep_params(W, b, a)
    ones = np.ones((1, P), np.float32)
    ident = np.eye(P, dtype=np.float32)
    return [
        {"x": np.ascontiguousarray(x[i]), "wf": wf, "bias": bias, "cb08": cb08,
         "ones": ones, "ident": ident}
        for i in range(NCORES)
    ]


_PROGRAM = None


def kernel(x, W, b, a):
    global _PROGRAM
    from concourse import bass_utils

    x = np.asarray(x, np.float32)
    assert x.shape == (B, N, IN), x.shape

    if _PROGRAM is None:
        _PROGRAM = _build_program()
    nc = _PROGRAM

    in_maps = _make_in_maps(x, W, b, a)
    res = bass_utils.run_bass_kernel_spmd(nc, in_maps, core_ids=list(range(NCORES)))
    out = np.stack([res.results[i]["out"] for i in range(NCORES)], axis=0)
    return out.astype(np.float32)


# revision 62
# speedup vs baseline: 395.2427x; 11.6061x over previous
"""GAT layer kernel for Trainium2 (Bass/Tile), SPMD over 8 NeuronCores.

Problem (fixed shapes, fp32):
    x: [8, 2048, 128], W: [4, 128, 64], b: [4, 64], a: [4, 128]
    h    = x @ W + b                    (per head)          [B,H,N,64]
    e    = leaky_relu(f_i[:,None] + f_j[None,:], 0.2)       [B,H,N,N]
    attn = softmax(e, axis=-1)
    out  = mean_h(attn @ h)                                 [B,N,64]
  where f_i = h @ a1, f_j = h @ a2.

Sharding: data-parallel — one batch element per core (B == 8 == n_cores).

Math used on-device (exact reformulation):
  exp(leaky(s)) = max(exp(s), exp(0.2 s))  (exp is monotone).
  Softmax over j is invariant to any per-row (per-i) positive scale, so divide
  row i by exp(0.2*c_i):
      Z[j,i] = max( exp(0.8*c_i) * exp(g_j), exp(0.2*g_j) )
  with c = f_i, g = f_j.  Then
      out[i,:] = (sum_j Z[j,i] h[j,:]) / (sum_j Z[j,i]).
  Z is ONE vector-engine tensor_scalar op per tile:
      Z = (v08_bcast * u1[j]) max u2[j]
  where v08_bcast[p, i] = exp(0.8*c_i) broadcast along partitions,
  u1 = exp(g), u2 = exp(0.2*g) are per-partition scalars.

Per core the attention product is computed transposed on the PE:
      oT[o, i] += h_ones[j, o].T @ Z[j, i]   accumulated over j tiles,
  where h_ones = [h_head | 4.0] so row 64 of oT is 4*denominator (the 4 bakes
  in the mean over the 4 heads).  oT is transposed back with the PE, then
  normalized with a reciprocal + per-partition-scalar multiply.
"""

import os
import sys

import numpy as np

_TRN_REPO = "/opt/trn_rl_repo"
if _TRN_REPO not in sys.path and os.path.isdir(_TRN_REPO):
    sys.path.insert(0, _TRN_REPO)

B, N, IN, OUT, H = 8, 2048, 128, 64, 4
NEG_SLOPE = 0.2
NCORES = 8
P = 128  # partition tile

# Column layout of the fused weight matrix WF [IN, H*(OUT+1) + 2*H]:
#   cols h*(OUT+1) .. h*(OUT+1)+OUT-1 : W[h]          -> h values
#   col  h*(OUT+1)+OUT                : zeros (bias 4.0) -> constant 4.0 column
#   col  H*(OUT+1) + h                : W[h] @ a1[h]  -> c = f_i per head
#   col  H*(OUT+1) + H + h            : W[h] @ a2[h]  -> g = f_j per head
HO = OUT + 1          # 65
CBASE = H * HO        # 260
GBASE = H * HO + H    # 264
WCOLS = H * HO + 2 * H  # 268


def _build_program(n=N, attn_f32r=True, repeat=1, hw_loop=0, z_gpsimd=False,
                   dma_bcast=False, bcast_f32r=True, z_bf16=False, z_fp16=True,
                   hint_engines=False, z_bufs=4, vbc_bufs=2, oTsb_bufs=2,
                   hgen_f32r=True, pb_bcast=True):
    import concourse.bass as bass
    import concourse.tile as tile
    from concourse import bacc, mybir

    f32 = mybir.dt.float32
    f32r = mybir.dt.float32r
    bf16 = mybir.dt.bfloat16
    T = n // P          # node tiles (16)
    IBS = min(512, n)   # i-block size for attn matmuls (one PSUM bank)
    NIB = n // IBS      # i blocks (4)
    f16 = mybir.dt.float16
    if z_bf16:
        hw_dt = bf16
    elif z_fp16:
        hw_dt = f16
    else:
        hw_dt = f32r if attn_f32r else f32
    z_dt = hw_dt
    vb_dt = hw_dt if (z_bf16 or z_fp16) else f32

    nc = bacc.Bacc("TRN2", target_bir_lowering=False, debug=False)

    x_d = nc.dram_tensor("x", [n, IN], f32, kind="ExternalInput")
    wf_d = nc.dram_tensor("wf", [IN, WCOLS], f32, kind="ExternalInput")
    bias_d = nc.dram_tensor("bias", [1, WCOLS], f32, kind="ExternalInput")
    cb08_d = nc.dram_tensor("cb08", [1, H], f32, kind="ExternalInput")
    ones_d = nc.dram_tensor("ones", [1, P], f32, kind="ExternalInput")
    ident_d = nc.dram_tensor("ident", [P, P], f32, kind="ExternalInput")
    out_d = nc.dram_tensor("out", [n, OUT], f32, kind="ExternalOutput")
    # internal DRAM bounce buffer for the partition-broadcast of exp(0.8c)
    e08s_d = nc.dram_tensor("e08scratch", [1, H * n], f32)

    Exp = mybir.ActivationFunctionType.Exp
    mult = mybir.AluOpType.mult
    amax = mybir.AluOpType.max
    add = mybir.AluOpType.add

    bc_dt = f32r if bcast_f32r else f32

    def body(tc, cst, rep):
        (wf_sb, bias_sb, ones_sb, cb08_sb, ident_sb, ones_r, wfc_r,
         wfr_sb, biasr_sb, ones_v) = cst
        with tc.tile_pool(name="bigbuf", bufs=1) as bigpool:
            # x in natural layout, tiled: [128, T*128]; col t*128+i = x[t*128+p, i]
            # (one DMA per tile so the transposes can start early)
            x_sb = bigpool.tile([P, T * IN], f32, tag="x")
            for t in range(T):
                nc.sync.dma_start(
                    x_sb[:, t * IN : (t + 1) * IN],
                    x_d.ap()[t * P : (t + 1) * P, :],
                )

            # ---- transpose x tiles, compute h (+f columns) per node tile ----
            # rounded copy of the [h | 4.0] weight blocks for the
            # attention matmuls (verifier requires rounded producers)
            hw_sb = bigpool.tile([P, T * CBASE], hw_dt, tag="hw")
            u1_sb = bigpool.tile([P, T * H], f32, tag="u1")  # exp(g)
            u2_sb = bigpool.tile([P, T * H], f32, tag="u2")  # exp(0.2 g)
            # per-head rows [1, n] of exp(0.8 * c), all on partition 0
            e08_dt = vb_dt if (pb_bcast or z_bf16 or z_fp16) else bc_dt
            e08row_sb = bigpool.tile([1, H * n], e08_dt, tag="e08row")
            # f32r copy of xT for the (cheap, 1 cyc/col) c-row matmuls
            xTr_sb = bigpool.tile([P, T * P], bc_dt, tag="xTr")
            if not hgen_f32r:
                xT_sb = bigpool.tile([P, T * P], f32, tag="xT")  # [i, n]

            with tc.tile_pool(name="setup_ps", bufs=2, space="PSUM") as spool:
                for t in range(T):
                    ps = spool.tile([P, P], f32, tag="xtr")
                    nc.tensor.transpose(
                        ps[:], x_sb[:, t * IN : (t + 1) * IN], ident_sb[:]
                    )
                    if not hgen_f32r:
                        nc.scalar.copy(xT_sb[:, t * P : (t + 1) * P], ps[:])
                    nc.scalar.copy(xTr_sb[:, t * P : (t + 1) * P], ps[:])
                if hgen_f32r:
                    hx_sb, hwf_sb, hbias_sb = xTr_sb, wfr_sb, biasr_sb
                else:
                    hx_sb, hwf_sb, hbias_sb = xT_sb, wf_sb, bias_sb
                hones_sb = ones_r if hgen_f32r else ones_sb
                for t in range(T):
                    ps = spool.tile([P, WCOLS], f32, tag="hmm")
                    # bias broadcast (K=1 matmul), then x.T @ WF accumulated
                    nc.tensor.matmul(
                        ps[:], hones_sb[:], hbias_sb[:], start=True, stop=False
                    )
                    nc.tensor.matmul(
                        ps[:],
                        hx_sb[:, t * P : (t + 1) * P],
                        hwf_sb[:],
                        start=False,
                        stop=True,
                    )
                    nc.scalar.copy(
                        hw_sb[:, t * CBASE : (t + 1) * CBASE], ps[:, 0:CBASE]
                    )
                    gcols = ps[:, GBASE : GBASE + H]
                    nc.scalar.activation(
                        u1_sb[:, t * H : (t + 1) * H], gcols, Exp, scale=1.0
                    )
                    nc.scalar.activation(
                        u2_sb[:, t * H : (t + 1) * H], gcols, Exp, scale=0.2
                    )

                # ---- per-head row of exp(0.8*c): c_row = x @ w1_h (M=1) ----
                for h in range(H):
                    for ib in range(NIB):
                        psc = spool.tile([1, IBS], f32, tag="crow")
                        nc.tensor.matmul(
                            psc[:],
                            wfc_r[:, h : h + 1],
                            xTr_sb[:, ib * IBS : (ib + 1) * IBS],
                            start=True,
                            stop=True,
                        )
                        nc.scalar.activation(
                            e08row_sb[0:1, h * n + ib * IBS : h * n + (ib + 1) * IBS],
                            psc[:],
                            Exp,
                            scale=0.8,
                            bias=cb08_sb[0:1, h : h + 1],
                        )

            if dma_bcast:
                # bounce exp(0.8c) rows through DRAM so they can be
                # DMA-broadcast across partitions (stride-0 reads are only
                # legal on DRAM APs) -- measured slower than the matmul
                # broadcast, kept for reference
                nc.sync.dma_start(e08s_d.ap(), e08row_sb[:].bitcast(f32))

            acc_sb = bigpool.tile([P, T * OUT], f32, tag="acc")

            with (
                tc.tile_pool(name="oT_ps", bufs=4, space="PSUM") as oTp,
                tc.tile_pool(name="tr_ps", bufs=2, space="PSUM") as trp,
                tc.tile_pool(name="vbc", bufs=vbc_bufs) as vbcpool,
                tc.tile_pool(name="z", bufs=z_bufs) as zpool,
                tc.tile_pool(name="oTsb", bufs=oTsb_bufs) as oTsbpool,
                tc.tile_pool(name="small", bufs=8) as smallpool,
            ):
                for h in range(H):
                    # ---- broadcast exp(0.8 c_h) across partitions ----
                    v08bc = vbcpool.tile([P, n], vb_dt, tag="v08bc")
                    if pb_bcast:
                        nc.gpsimd.partition_broadcast(
                            v08bc[:], e08row_sb[0:1, h * n : (h + 1) * n]
                        )
                    for ib in range(NIB if not pb_bcast else 0):
                        if dma_bcast:
                            sl = e08s_d.ap()[0:1, h * n + ib * IBS : h * n + (ib + 1) * IBS]
                            bcast_ap = bass.AP(
                                tensor=sl.tensor, offset=sl.offset,
                                ap=[[0, P]] + sl.ap[1:],
                            )
                            nc.sync.dma_start(
                                v08bc[:, ib * IBS : (ib + 1) * IBS], bcast_ap
                            )
                        else:
                            psb = trp.tile([P, IBS], f32, tag="vbc")
                            nc.tensor.matmul(
                                psb[:],
                                ones_v[:],
                                e08row_sb[0:1, h * n + ib * IBS : h * n + (ib + 1) * IBS],
                                start=True,
                                stop=True,
                            )
                            nc.scalar.copy(v08bc[:, ib * IBS : (ib + 1) * IBS], psb[:])

                    # ---- attention: oT[o, i] += h_ones.T @ Z over j tiles ----
                    oT_ps = [
                        oTp.tile([HO, IBS], f32, tag="oT", name=f"oT_{rep}_{h}_{ib}")
                        for ib in range(NIB)
                    ]
                    for jt in range(T):
                        z = zpool.tile([P, n], z_dt, tag="z")
                        # split the big elementwise pass between DVE and GPSIMD
                        zeng = nc.gpsimd if (z_gpsimd and jt % 3 == 2) else nc.vector
                        zeng.tensor_scalar(
                            z[:],
                            v08bc[:],
                            u1_sb[:, jt * H + h : jt * H + h + 1],
                            u2_sb[:, jt * H + h : jt * H + h + 1],
                            op0=mult,
                            op1=amax,
                        )
                        lhs = hw_sb[:, jt * CBASE + h * HO : jt * CBASE + (h + 1) * HO]
                        for ib in range(NIB):
                            nc.tensor.matmul(
                                oT_ps[ib][:],
                                lhs,
                                z[:, ib * IBS : (ib + 1) * IBS],
                                start=(jt == 0),
                                stop=(jt == T - 1),
                            )

                    # ---- transpose back, normalize, accumulate over heads ----
                    oT_sb = oTsbpool.tile([HO, n], f32, tag="oTsb")
                    for ib in range(NIB):
                        nc.scalar.copy(
                            oT_sb[:, ib * IBS : (ib + 1) * IBS], oT_ps[ib][:]
                        )
                    for it in range(T):
                        pst = trp.tile([P, HO], f32, tag="otr")
                        nc.tensor.transpose(
                            pst[:],
                            oT_sb[:, it * P : (it + 1) * P],
                            ident_sb[0:HO, 0:HO],
                        )
                        rec = smallpool.tile([P, 1], f32, tag="rec")
                        nc.vector.reciprocal(rec[:], pst[:, OUT : OUT + 1])
                        accsl = acc_sb[:, it * OUT : (it + 1) * OUT]
                        if h == 0:
                            nc.vector.tensor_scalar(
                                accsl, pst[:, 0:OUT], rec[:], None, op0=mult
                            )
                        else:
                            nc.vector.scalar_tensor_tensor(
                                accsl, pst[:, 0:OUT], rec[:], accsl,
                                op0=mult, op1=add,
                            )

            # per-tile output DMAs so stores overlap the tail of the compute
            for t in range(T):
                nc.sync.dma_start(
                    out_d.ap()[t * P : (t + 1) * P, :],
                    acc_sb[:, t * OUT : (t + 1) * OUT],
                )

    with tile.TileContext(nc) as tc:
        with tc.tile_pool(name="const", bufs=1) as cpool:
            wf_sb = cpool.tile([IN, WCOLS], f32, tag="wf")
            nc.sync.dma_start(wf_sb[:], wf_d.ap())
            bias_sb = cpool.tile([1, WCOLS], f32, tag="bias")
            nc.sync.dma_start(bias_sb[:], bias_d.ap())
            ones_sb = cpool.tile([1, P], f32, tag="ones")
            nc.sync.dma_start(ones_sb[:], ones_d.ap())
            cb08_sb = cpool.tile([1, H], f32, tag="cb08")
            nc.sync.dma_start(cb08_sb[:], cb08_d.ap())
            ident_sb = cpool.tile([P, P], f32, tag="ident")
            nc.sync.dma_start(ident_sb[:], ident_d.ap())
            ones_r = cpool.tile([1, P], bc_dt, tag="ones_r")
            nc.vector.tensor_copy(ones_r[:], ones_sb[:])
            wfc_r = cpool.tile([IN, H], bc_dt, tag="wfc_r")
            nc.vector.tensor_copy(wfc_r[:], wf_sb[:, CBASE : CBASE + H])
            wfr_sb = cpool.tile([IN, WCOLS], bc_dt, tag="wfr")
            nc.vector.tensor_copy(wfr_sb[:], wf_sb[:])
            biasr_sb = cpool.tile([1, WCOLS], bc_dt, tag="biasr")
            nc.vector.tensor_copy(biasr_sb[:], bias_sb[:])
            e08_dt_ = vb_dt if (pb_bcast or z_bf16 or z_fp16) else bc_dt
            ones_v = cpool.tile([1, P], e08_dt_, tag="ones_v")
            nc.vector.tensor_copy(ones_v[:], ones_sb[:])

            cst = (wf_sb, bias_sb, ones_sb, cb08_sb, ident_sb, ones_r, wfc_r,
                   wfr_sb, biasr_sb, ones_v)
            if hw_loop:
                # hardware loop: body emitted once, looped on-device (used
                # for amortized timing measurements)
                hints = (
                    (mybir.EngineType.PE, mybir.EngineType.DVE,
                     mybir.EngineType.Activation)
                    if hint_engines else ()
                )
                with tc.For_i(0, hw_loop, 1, hint_engines=hints):
                    body(tc, cst, 0)
            else:
                for rep in range(repeat):
                    body(tc, cst, rep)

    nc.compile()
    return nc


def _prep_params(W, b, a):
    W = np.asarray(W, np.float32)
    b = np.asarray(b, np.float32)
    a = np.asarray(a, np.float32)
    a1, a2 = a[:, :OUT], a[:, OUT:]
    wf = np.zeros((IN, WCOLS), np.float32)
    bias = np.zeros((1, WCOLS), np.float32)
    cb08 = np.zeros((1, H), np.float32)
    for h in range(H):
        wf[:, h * HO : h * HO + OUT] = W[h]
        bias[0, h * HO : h * HO + OUT] = b[h]
        bias[0, h * HO + OUT] = float(H)  # denominator scale -> head mean
        wf[:, CBASE + h] = W[h] @ a1[h]
        bias[0, CBASE + h] = float(b[h] @ a1[h])
        wf[:, GBASE + h] = W[h] @ a2[h]
        bias[0, GBASE + h] = float(b[h] @ a2[h])
        cb08[0, h] = 0.8 * float(b[h] @ a1[h])
    return wf, bias, cb08


def _make_in_maps(x, W, b, a):
    wf, bias, cb08 = _prep_params(W, b, a)
    ones = np.ones((1, P), np.float32)
    ident = np.eye(P, dtype=np.float32)
    return [
        {"x": np.ascontiguousarray(x[i]), "wf": wf, "bias": bias, "cb08": cb08,
         "ones": ones, "ident": ident}
        for i in range(NCORES)
    ]


_PROGRAM = None


def kernel(x, W, b, a):
    global _PROGRAM
    from concourse import bass_utils

    x = np.asarray(x, np.float32)
    assert x.shape == (B, N, IN), x.shape

    if _PROGRAM is None:
        _PROGRAM = _build_program()
    nc = _PROGRAM

    in_maps = _make_in_maps(x, W, b, a)
    res = bass_utils.run_bass_kernel_spmd(nc, in_maps, core_ids=list(range(NCORES)))
    out = np.stack([res.results[i]["out"] for i in range(NCORES)], axis=0)
    return out.astype(np.float32)
